# revision 1
# baseline (speedup 1.0000x reference)
"""GAT layer (dense adjacency) on 8 Trainium2 NeuronCores.

Problem: H = elu(softmax_j(mask(A, leaky_relu(Wh1_i + Wh2_j))) @ Wh),
A: [8, 2048, 2048] 0/1 f32, X: [8, 2048, 64], Ws: [64, 64], a: [128, 1].

Sharding: data-parallel over batch B=8 -> one batch element per core.

Per-core device algorithm (transposed layout, rows = source node j,
cols = destination node i):
  - Host precomputes Wh = X@Ws, Wh1 = Wh@a1, Wh2 = Wh@a2 (tiny: 0.1% of work).
  - For each j-tile (128 rows) the masked logits are built in PSUM by the
    tensor engine itself:
        P[j, i] = ones2^T @ [Wh1_hi; Wh1_lo]  (broadcast of Wh1 along j)
                + (A_block)^T @ (C*I)         (mask: C=512 where edge, 0 else)
    so no elementwise mask multiply and no separate transpose of the
    attention matrix is ever needed (the matmul with C*I transposes A).
  - ACT: e = LeakyRelu(P + (Wh2[j] - C)), pa = Exp(e - S) -> fp16.
    Where A=0 the logit is z - 512 -> exp ~ 0.  Where A=1 it is exactly z.
    S is a host-computed shift keeping pa in fp16 range; softmax scale
    invariance cancels it.
  - H^T[d, i] (+ row sums s_i via an appended ones column) accumulate on the
    tensor engine in fp16: H^T = sum_t WhAug_t^T @ pa_t.
  - Epilogue: PE-transpose H^T 128-col chunks back to [128, 65], 1/s via DVE
    reciprocal, H = elu(H_pre * (1/s)) built from Exp/min/relu ops.
Optionally a subset of tiles computes the LeakyRelu on the vector engine
(3 tensor_scalar/tensor_tensor ops) to balance ACT vs DVE.
"""
import sys

for _p in ("/opt/trn_rl_repo",):
    if _p not in sys.path:
        sys.path.append(_p)

import numpy as np
import ml_dtypes

import concourse.bass as bass
import concourse.bacc as bacc
import concourse.tile as tile
from concourse import mybir
from concourse import bass_utils

F32 = mybir.dt.float32
BF16 = mybir.dt.bfloat16
FP16 = mybir.dt.float16
AF = mybir.ActivationFunctionType
ALU = mybir.AluOpType

B, N, F, D = 8, 2048, 64, 64
NT = N // 128          # 16 j-tiles / i-tiles
HALF = N // 2          # 1024 columns processed per psum tile
C_MASK = 512.0
ALPHA = 0.2
# (h, t) pairs whose LeakyRelu runs on DVE instead of ACT (engine balance).
DVE_LR = {(h, t) for h in range(2) for t in range(NT) if t % 2 == 0}

_CACHED = {}


def _build_program():
    nc = bacc.Bacc("TRN2", target_bir_lowering=False, debug=False)

    A_d = nc.dram_tensor("A", [N, N], F32, kind="ExternalInput")
    whaug_d = nc.dram_tensor("WhAug16", [N, D + 1], FP16, kind="ExternalInput")
    wh1p_d = nc.dram_tensor("wh1p", [2, N], BF16, kind="ExternalInput")
    biasT_d = nc.dram_tensor("biasT", [128, NT], F32, kind="ExternalInput")
    negS_d = nc.dram_tensor("negS", [128, 1], F32, kind="ExternalInput")
    ci_d = nc.dram_tensor("CI", [128, 128], BF16, kind="ExternalInput")
    ones2_d = nc.dram_tensor("ones2", [2, 128], BF16, kind="ExternalInput")
    i65_d = nc.dram_tensor("I65", [D + 1, D + 1], F32, kind="ExternalInput")
    H_d = nc.dram_tensor("H", [N, D], F32, kind="ExternalOutput")

    with tile.TileContext(nc) as tc:
        with tc.tile_pool(name="const", bufs=1) as cp, \
             tc.tile_pool(name="aslab", bufs=NT) as ap_pool, \
             tc.tile_pool(name="work", bufs=3) as wp, \
             tc.tile_pool(name="outp", bufs=1) as op_pool, \
             tc.tile_pool(name="psP", bufs=2, space="PSUM") as psP, \
             tc.tile_pool(name="psH", bufs=1, space="PSUM") as psH, \
             tc.tile_pool(name="psT", bufs=2, space="PSUM") as psT:

            # ---- constants / small inputs ----
            whaug = cp.tile([128, NT * (D + 1)], FP16, name="whaug")
            nc.sync.dma_start(
                whaug[:].rearrange("p (t c) -> p t c", c=D + 1),
                whaug_d.ap().rearrange("(t p) c -> p t c", p=128),
            )
            wh1p = cp.tile([2, N], BF16, name="wh1p")
            nc.sync.dma_start(wh1p[:], wh1p_d.ap())
            biasT = cp.tile([128, NT], F32, name="biasT")
            nc.sync.dma_start(biasT[:], biasT_d.ap())
            negS = cp.tile([128, 1], F32, name="negS")
            nc.sync.dma_start(negS[:], negS_d.ap())
            ci = cp.tile([128, 128], BF16, name="ci")
            nc.sync.dma_start(ci[:], ci_d.ap())
            ones2 = cp.tile([2, 128], BF16, name="ones2")
            nc.sync.dma_start(ones2[:], ones2_d.ap())
            i65 = cp.tile([D + 1, D + 1], F32, name="i65")
            nc.sync.dma_start(i65[:], i65_d.ap())
            alpha02 = cp.tile([128, 1], F32, name="alpha02")
            nc.vector.memset(alpha02[:], ALPHA)
            # preload ACT table set (Exp/Prelu share one set) during input DMA
            warm = cp.tile([1, 1], F32, name="warm")
            nc.vector.memset(warm[:], 0.0)
            warm2 = cp.tile([1, 1], F32, name="warm2")
            nc.scalar.activation(warm2[:], warm[:], AF.Exp, bias=0.0, scale=1.0)

            # ---- A column-slabs, f32 via HWDGE; the mask matmuls read a
            # stride-2 bf16 view (A is exactly 0/1, so the f32 high halves
            # ARE the bf16 values - no cast DMA needed) ----
            # aslab[t][p, 128*r + q] = A[128*r + p, 128*t + q]
            aslabs = []
            for t in range(NT):
                sl = ap_pool.tile([128, N], F32, name=f"aslab{t}", tag="aslab")
                nc.sync.dma_start(
                    sl[:].rearrange("p (r q) -> p r q", q=128),
                    A_d.ap()[:, 128 * t:128 * (t + 1)]
                    .rearrange("(r p) q -> p r q", p=128),
                )
                aslabs.append(sl)

            hout = op_pool.tile([128, NT * D], F32, name="hout")

            for h in range(2):
                ht_ps = psH.tile([D + 1, HALF], F32, name="ht_ps", tag="ht_ps")

                def fill(t, h=h):
                    pp = psP.tile([128, HALF], F32, name="pp", tag="pp")
                    # broadcast of Wh1 (hi+lo rows, K=2) along partitions
                    for c in range(2):
                        nc.tensor.matmul(
                            pp[:, 512 * c:512 * (c + 1)],
                            ones2[:],
                            wh1p[:, HALF * h + 512 * c:HALF * h + 512 * (c + 1)],
                            start=True, stop=False,
                        )
                    # mask + transpose: pp[:, 128k:+128] += C * A_block^T
                    for k in range(8):
                        r = 8 * h + k
                        nc.tensor.matmul(
                            pp[:, 128 * k:128 * (k + 1)],
                            aslabs[t][:].bitcast(BF16)[:, 256 * r + 1:256 * (r + 1):2],
                            ci[:],
                            start=False, stop=True, skip_group_check=True,
                        )
                    return pp

                pp_next = fill(0)
                for t in range(NT):
                    pp = pp_next
                    if t + 1 < NT:
                        pp_next = fill(t + 1)
                    pa = wp.tile([128, HALF], FP16, name="pa", tag="pa")
                    if (h, t) in DVE_LR:
                        # LeakyRelu on DVE: z = pp + bias; e = max(z, 0.2 z)
                        z1 = wp.tile([128, HALF], F32, name="z1", tag="z1")
                        nc.vector.tensor_scalar(
                            z1[:], pp[:], biasT[:, t:t + 1], None, ALU.add)
                        z2 = wp.tile([128, HALF], F32, name="z2", tag="z2")
                        nc.vector.tensor_scalar(
                            z2[:], z1[:], ALPHA, None, ALU.mult)
                        e_t = wp.tile([128, HALF], F32, name="e_t", tag="e")
                        nc.vector.tensor_tensor(e_t[:], z1[:], z2[:], ALU.max)
                    else:
                        # LeakyRelu on ACT with per-partition bias
                        e_t = wp.tile([128, HALF], F32, name="e_t", tag="e")
                        nc.scalar.activation(
                            e_t[:], pp[:], AF.Prelu,
                            bias=biasT[:, t:t + 1], scale=1.0, alpha=alpha02[:])
                    nc.scalar.activation(
                        pa[:], e_t[:], AF.Exp, bias=negS[:], scale=1.0)
                    # H^T accumulation (+ ones column -> row sums)
                    for c in range(2):
                        nc.tensor.matmul(
                            ht_ps[:, 512 * c:512 * (c + 1)],
                            whaug[:, (D + 1) * t:(D + 1) * (t + 1)],
                            pa[:, 512 * c:512 * (c + 1)],
                            start=(t == 0), stop=(t == NT - 1),
                        )
                # ---- epilogue for this half ----
                ht_sb = wp.tile([D + 1, HALF], F32, name="ht_sb", tag="ht_sb", bufs=2)
                nc.vector.tensor_copy(ht_sb[:], ht_ps[:])
                for k in range(8):
                    t2 = 8 * h + k
                    tr = psT.tile([128, D + 1], F32, name="tr", tag="tr")
                    nc.tensor.matmul(
                        tr[:], ht_sb[:, 128 * k:128 * (k + 1)], i65[:],
                        is_transpose=True, start=True, stop=True)
                    rc = wp.tile([128, 1], F32, name="rc", tag="rc", bufs=4)
                    nc.vector.reciprocal(rc[:], tr[:, D:D + 1])
                    # elu(x*r) = relu(x*r) + min(exp(x*r) - 1, 0)
                    w_t = wp.tile([128, D], F32, name="w_t", tag="w_t")
                    nc.scalar.activation(
                        w_t[:], tr[:, 0:D], AF.Exp, bias=0.0, scale=rc[:])
                    q_t = wp.tile([128, D], F32, name="q_t", tag="q_t")
                    nc.vector.tensor_scalar(
                        q_t[:], w_t[:], -1.0, 0.0, ALU.add, ALU.min)
                    r2 = wp.tile([128, D], F32, name="r2", tag="r2")
                    nc.vector.tensor_scalar(
                        r2[:], tr[:, 0:D], rc[:], 0.0, ALU.mult, ALU.max)
                    nc.vector.tensor_tensor(
                        hout[:, D * t2:D * (t2 + 1)], q_t[:], r2[:], ALU.add)
                nc.sync.dma_start(
                    H_d.ap()[1024 * h:1024 * (h + 1), :]
                    .rearrange("(t p) d -> p t d", p=128),
                    hout[:, 8 * D * h:8 * D * (h + 1)]
                    .rearrange("p (t d) -> p t d", d=D),
                )



    nc.compile()
    return nc


def _get_program():
    if "nc" not in _CACHED:
        _CACHED["nc"] = _build_program()
    return _CACHED["nc"]


def _host_prep(A, X, Ws, a):
    """Per-core host-side input preparation (cheap: ~67 MFLOP total)."""
    f64 = np.float64
    in_maps = []
    for b in range(B):
        Wh = X[b].astype(f64) @ Ws.astype(f64)            # [N, D]
        Wh1 = (Wh @ a[:D].astype(f64))[:, 0]              # [N]
        Wh2 = (Wh @ a[D:].astype(f64))[:, 0]              # [N]
        S = max(0.0, float(Wh1.max() + Wh2.max()) - 10.5)
        whaug = np.ones((N, D + 1), np.float16)
        whaug[:, :D] = Wh.astype(np.float16)
        wh1_hi = Wh1.astype(ml_dtypes.bfloat16)
        wh1_lo = (Wh1 - wh1_hi.astype(f64)).astype(ml_dtypes.bfloat16)
        wh1p = np.stack([wh1_hi, wh1_lo])                  # [2, N]
        biasT = (Wh2 - C_MASK).astype(np.float32).reshape(NT, 128).T.copy()
        in_maps.append({
            "A": np.ascontiguousarray(A[b]),
            "WhAug16": whaug,
            "wh1p": wh1p,
            "biasT": np.ascontiguousarray(biasT),
            "negS": np.full((128, 1), -S, np.float32),
            "CI": (C_MASK * np.eye(128)).astype(ml_dtypes.bfloat16),
            "ones2": np.ones((2, 128), ml_dtypes.bfloat16),
            "I65": np.eye(D + 1, dtype=np.float32),
        })
    return in_maps


def kernel(A, X, Ws, a, _trace=False, _trace_kwargs=None):
    A = np.asarray(A, np.float32)
    X = np.asarray(X, np.float32)
    Ws = np.asarray(Ws, np.float32)
    a = np.asarray(a, np.float32)
    nc = _get_program()
    in_maps = _host_prep(A, X, Ws, a)
    kw = {}
    if _trace:
        kw = {"trace": True, **(_trace_kwargs or {})}
    res = bass_utils.run_bass_kernel_spmd(nc, in_maps, core_ids=list(range(B)), **kw)
    H = np.stack([np.asarray(res.results[b]["H"]) for b in range(B)])
    if _trace:
        kernel.last_results = res
    return H



# revision 31
# speedup vs baseline: 1.4268x; 1.4268x over previous
"""GAT layer (dense adjacency) on 8 Trainium2 NeuronCores.

Problem: H = elu(softmax_j(mask(A, leaky_relu(Wh1_i + Wh2_j))) @ Wh),
A: [8, 2048, 2048] 0/1 f32, X: [8, 2048, 64], Ws: [64, 64], a: [128, 1].

Sharding: data-parallel over batch B=8 -> one batch element per core.

Per-core device algorithm (layout: rows = source j, cols = destination i):
  - Host precomputes Wh = X@Ws, Wh1 = Wh@a1, Wh2 = Wh@a2 (tiny) and packs
    A as fp8-e4m3 slabs with embedded bias rows.
  - Logits built by fp8 DoubleRow matmuls (0.5 cyc/row), one [128,128]
    chunk per matmul with TWO k-slots:
      slot0: lhsT = A-block [i,j],  rhs = C*I      -> C*A^T  (mask+transpose)
      slot1: lhsT = bias rows,      rhs = wh1/ones -> Wh1_i + (Wh2_j - C)
    so PSUM gets  z - C*(1-A)  directly (z = Wh1_i + Wh2_j), C = 192.
  - leaky_relu in ONE op on DVE/Pool: e = (pp * 0.2) max pp
    (scalar_tensor_tensor); masked entries stay ~ z - C -> exp ~= 0.
  - exp on ACT only, wide instructions: pa = Exp(e - S) -> fp16.
  - H^T[d, i] (+ row sums s_i via an appended ones column) accumulates on
    the tensor engine in fp16 into a [65, 2048] PSUM tile.
  - Device ships Hpre = [65, 2048] (numerators + sums); the host does the
    final divide + elu + transpose (1M elements, ~0.01% of the FLOPs).
"""
import sys

for _p in ("/opt/trn_rl_repo",):
    if _p not in sys.path:
        sys.path.append(_p)

import numpy as np
import ml_dtypes

import concourse.bass as bass
import concourse.bacc as bacc
import concourse.tile as tile
from concourse import mybir
from concourse import bass_utils

F32 = mybir.dt.float32
FP16 = mybir.dt.float16
F8 = mybir.dt.float8e4
AF = mybir.ActivationFunctionType
ALU = mybir.AluOpType
DR = mybir.MatmulPerfMode.DoubleRow
E4 = ml_dtypes.float8_e4m3

B, N, F, D = 8, 2048, 64, 64
NT = N // 128            # 16 j-tiles
C_MASK = 192.0           # mask offset; exactly representable in e4m3
ALPHA = 0.2
NW1, NW2 = 4, 6          # fp8 split counts for Wh1 / (Wh2 - C)
SLAB_W = (NT + 1) * 128  # 17 blocks of 128: 16 A blocks + 1 bias block
# exp group sizes in j-tiles per half: small first group (pipeline spin-up)
# and last group (short tail).
EGROUPS = [2, 4, 4, 4, 2]
# leaky-relu: hardware allows only ONE PSUM source per vector op and no
# GPSIMD access to PSUM at all, so the options per [128, 512] chunk are:
#   'A': ACT Prelu straight from PSUM (1 op, ~570 ns, shares ACT with exp)
#   'D': DVE 2-op (mult to SBUF scratch, then max(psum, sbuf)) ~1316 ns
#   'P': DVE fp16 copy (~658) + Pool 2-op in SBUF (~1612 ns)
# Shares 21/18/25 balance ACT(+exp)/DVE/Pool busy; smooth-interleaved so the
# in-order PE fill stream matches consumption order.
def _mk_kinds():
    shares = {"A": 29 / 64, "D": 35 / 64}
    acc = {k: 0.0 for k in shares}
    out = []
    for _ in range(64):
        for k in shares:
            acc[k] += shares[k]
        pick = max(acc, key=lambda k: acc[k])
        acc[pick] -= 1.0
        out.append(pick)
    return out


_KINDS = _mk_kinds()

_CACHED = {}


def _build_program():
    nc = bacc.Bacc("TRN2", target_bir_lowering=False, debug=False)

    ap_d = nc.dram_tensor("Apack", [NT * 128, SLAB_W], F8, kind="ExternalInput")
    rt_d = nc.dram_tensor("rhstab", [128, SLAB_W], F8, kind="ExternalInput")
    wh_d = nc.dram_tensor("whaugP", [128, NT * (D + 1)], FP16, kind="ExternalInput")
    ns_d = nc.dram_tensor("negS", [128, 1], F32, kind="ExternalInput")
    H_d = nc.dram_tensor("Hpre", [D + 1, N], F32, kind="ExternalOutput")

    with tile.TileContext(nc) as tc:
        with tc.tile_pool(name="const", bufs=1) as cp, \
             tc.tile_pool(name="aslab", bufs=NT) as apool, \
             tc.tile_pool(name="work", bufs=2) as wp, \
             tc.tile_pool(name="psA", bufs=2, space="PSUM") as psA, \
             tc.tile_pool(name="psD", bufs=2, space="PSUM") as psD, \
             tc.tile_pool(name="psG", bufs=2, space="PSUM") as psG, \
             tc.tile_pool(name="psH", bufs=1, space="PSUM") as psH:

            # ---- inputs. Parallel queues so the first fill isn't gated on a
            # serial SP DMA stream: slab0 on SP, rhstab on ACT's HWDGE, the
            # small negS/whaug on DVE's. Remaining slabs stream on SP. ----
            rhstab = cp.tile([128, SLAB_W], F8, name="rhstab")
            aslabs = [apool.tile([128, SLAB_W], F8, name=f"aslab{t}",
                                 tag="aslab") for t in range(NT)]
            negS = cp.tile([128, 1], F32, name="negS")
            whaug = cp.tile([128, NT * (D + 1)], FP16, name="whaug")
            # slab0 in three pieces so the first fill (A block 0 + bias
            # block) can start as early as possible
            nc.sync.dma_start(aslabs[0][:, 0:128], ap_d.ap()[0:128, 0:128])
            nc.sync.dma_start(aslabs[0][:, NT * 128:SLAB_W],
                              ap_d.ap()[0:128, NT * 128:SLAB_W])
            nc.scalar.dma_start(rhstab[:], rt_d.ap())
            nc.scalar.dma_start(negS[:], ns_d.ap())
            nc.sync.dma_start(aslabs[0][:, 128:NT * 128],
                              ap_d.ap()[0:128, 128:NT * 128])
            nc.scalar.dma_start(whaug[:], wh_d.ap())
            for t in range(1, NT):
                nc.sync.dma_start(aslabs[t][:],
                                  ap_d.ap()[128 * t:128 * (t + 1), :])
            # preload the Exp/Prelu table during input DMA
            warm = cp.tile([1, 1], F32, name="warm")
            nc.vector.memset(warm[:], 0.0)
            warm2 = cp.tile([1, 1], F32, name="warm2")
            nc.scalar.activation(warm2[:], warm[:], AF.Exp, bias=0.0, scale=1.0)
            alpha02 = cp.tile([128, 1], F32, name="alpha02")
            nc.vector.memset(alpha02[:], ALPHA)

            rh3 = rhstab[:].rearrange("p (x q) -> p x q", q=128)

            for h in range(2):
                ht = psH.tile([D + 1, 1024], F32, name="ht", tag="ht")
                pending_ht = []

                def emit_ht(group, ht=ht):
                    tstart, gsz, pa_t = group
                    for tt in range(tstart, tstart + gsz):
                        rel = 1024 * (tt - tstart)
                        for b2 in range(2):
                            nc.tensor.matmul(
                                ht[:, 512 * b2:512 * (b2 + 1)],
                                whaug[:, (D + 1) * tt:(D + 1) * (tt + 1)],
                                pa_t[:, rel + 512 * b2:rel + 512 * (b2 + 1)],
                                start=(tt == 0), stop=(tt == NT - 1),
                                skip_group_check=True)

                tstart = 0
                for gsz in EGROUPS:
                    e_t = wp.tile([128, 1024 * gsz], FP16, name=f"e{gsz}",
                                  tag=f"e{gsz}", bufs=3)
                    pa_t = wp.tile([128, 1024 * gsz], FP16, name=f"pa{gsz}",
                                   tag=f"pa{gsz}", bufs=3)
                    for tt in range(tstart, tstart + gsz):
                        as3 = aslabs[tt][:].rearrange("p (x q) -> p x q", q=128)
                        for k in range(2):
                            g = 32 * h + 2 * tt + k
                            kind = _KINDS[g]
                            pool = {"A": psA, "D": psD, "P": psG}[kind]
                            pp = pool.tile([128, 512], F32, name="pp",
                                           tag="pp" + kind)
                            for c4 in range(4):
                                c = 8 * h + 4 * k + c4
                                nc.tensor.matmul(
                                    pp[:, 128 * c4:128 * (c4 + 1)],
                                    as3[:, c:NT + 1:NT - c, :],
                                    rh3[:, 0:2 + c:1 + c, :],
                                    start=(c4 == 0), stop=(c4 == 3),
                                    perf_mode=DR, skip_group_check=True)
                            off = 1024 * (tt - tstart) + 512 * k
                            ev = e_t[:, off:off + 512]
                            if kind == "A":
                                nc.scalar.activation(
                                    ev, pp[:], AF.Prelu, bias=0.0,
                                    scale=1.0, alpha=alpha02[:])
                            elif kind == "D":
                                u = wp.tile([128, 512], F32, name="uD",
                                            tag="uD", bufs=3)
                                nc.vector.tensor_scalar(
                                    u[:], pp[:], ALPHA, None, ALU.mult)
                                nc.vector.tensor_tensor(
                                    ev, pp[:], u[:], ALU.max)
                            else:
                                u16 = wp.tile([128, 512], FP16, name="uP",
                                              tag="uP", bufs=3)
                                nc.vector.tensor_copy(u16[:], pp[:])
                                v16 = wp.tile([128, 512], FP16, name="vP",
                                              tag="vP", bufs=3)
                                nc.gpsimd.tensor_scalar(
                                    v16[:], u16[:], ALPHA, None, ALU.mult)
                                nc.gpsimd.tensor_tensor(
                                    ev, u16[:], v16[:], ALU.max)
                    nc.scalar.activation(pa_t[:], e_t[:], AF.Exp,
                                         bias=negS[:], scale=1.0)
                    pending_ht.append((tstart, gsz, pa_t))
                    if len(pending_ht) > 2:
                        emit_ht(pending_ht.pop(0))
                    tstart += gsz
                for grp in pending_ht:
                    emit_ht(grp)

                # ship this half's numerators + sums (divide/elu on host)
                hs = wp.tile([D + 1, 1024], F32, name="hs", tag="hs", bufs=2)
                nc.vector.tensor_copy(hs[:], ht[:])
                nc.sync.dma_start(
                    H_d.ap()[:, 1024 * h:1024 * (h + 1)], hs[:])

    nc.compile()
    return nc


def _get_program():
    if "nc" not in _CACHED:
        _CACHED["nc"] = _build_program()
    return _CACHED["nc"]


def _split_e4m3(v, n):
    """Greedy hi->lo fp8-e4m3 decomposition of v (f64)."""
    out = []
    r = np.asarray(v, np.float64).copy()
    for _ in range(n):
        s = r.astype(E4).astype(np.float64)
        out.append(s)
        r -= s
    return np.stack(out)


def _host_prep(A, X, Ws, a):
    f64 = np.float64
    in_maps = []
    shifts = []
    for b in range(B):
        Wh = X[b].astype(f64) @ Ws.astype(f64)            # [N, D]
        Wh1 = (Wh @ a[:D].astype(f64))[:, 0]              # [N]
        Wh2 = (Wh @ a[D:].astype(f64))[:, 0]              # [N]
        S = max(0.0, float(Wh1.max() + Wh2.max()) - 10.5)
        shifts.append(S)
        wh1s = _split_e4m3(Wh1, NW1)                      # [NW1, N]
        wh2s = _split_e4m3(Wh2 - C_MASK, NW2)             # [NW2, N]

        # Apack[t][k, 128c+m] = A[128c+k, 128t+m] (A block x=c) plus a bias
        # block at x=16. 1.0 -> e4m3 byte 0x38 (cheap uint8 path).
        Au8 = (A[b] != 0).astype(np.uint8) * np.uint8(0x38)
        At = np.ascontiguousarray(
            Au8.reshape(NT, 128, NT, 128).transpose(2, 1, 0, 3)
        ).reshape(NT, 128, NT * 128)                      # [t, k, 2048]
        biasb = np.zeros((NT, 128, 128), np.float32)
        biasb[:, 0:NW1, :] = 1.0
        for r in range(NW2):
            biasb[:, NW1 + r, :] = wh2s[r].reshape(NT, 128)
        bias8 = biasb.astype(E4).view(np.uint8)
        apack = np.concatenate([At, bias8], axis=2).reshape(NT * 128, SLAB_W)
        apack = apack.view(E4)

        # rhstab: x=0 -> C*I; x=1+c -> rows 0..NW1-1 = wh1 splits, then ones
        rt = np.zeros((128, NT + 1, 128), np.float32)
        rt[:, 0, :] = C_MASK * np.eye(128, dtype=np.float32)
        w1r = wh1s.reshape(NW1, NT, 128)                  # [r, c, n]
        for c in range(NT):
            rt[0:NW1, 1 + c, :] = w1r[:, c, :]
            rt[NW1:NW1 + NW2, 1 + c, :] = 1.0
        rhstab = rt.reshape(128, SLAB_W).astype(E4)

        whaugP = np.ones((128, NT, D + 1), np.float16)
        whaugP[:, :, :D] = Wh.reshape(NT, 128, D).transpose(1, 0, 2)
        in_maps.append({
            "Apack": np.ascontiguousarray(apack),
            "rhstab": rhstab,
            "whaugP": np.ascontiguousarray(whaugP.reshape(128, NT * (D + 1))),
            "negS": np.full((128, 1), -S, np.float32),
        })
    return in_maps, shifts


def kernel(A, X, Ws, a, _trace=False, _trace_kwargs=None):
    A = np.asarray(A, np.float32)
    X = np.asarray(X, np.float32)
    Ws = np.asarray(Ws, np.float32)
    a = np.asarray(a, np.float32)
    nc = _get_program()
    in_maps, _shifts = _host_prep(A, X, Ws, a)
    kw = {}
    if _trace:
        kw = {"trace": True, **(_trace_kwargs or {})}
    res = bass_utils.run_bass_kernel_spmd(nc, in_maps, core_ids=list(range(B)), **kw)
    Hs = []
    for b in range(B):
        Hpre = np.asarray(res.results[b]["Hpre"]).astype(np.float64)
        num = Hpre[:D, :]                  # [D, N] numerators (transposed)
        s = Hpre[D, :]                     # [N] softmax denominators
        Hn = (num / s).T                   # [N, D]
        H = np.where(Hn > 0, Hn, np.expm1(Hn))
        Hs.append(H.astype(np.float32))
    if _trace:
        kernel.last_results = res
    return np.stack(Hs)
